# revision 1
# baseline (speedup 1.0000x reference)
"""Multi-head self-attention (B=4, N=2048, D=1024, H=16) on 8 trn2 NeuronCores.

Sharding: 8 shards = (batch, query-half).  Core c handles batch c//2 and query
rows [(c%2)*1024, (c%2)*1024+1024).  Each core receives its batch's z with the
rows rolled so that its query rows come first; rolling permutes the key/value
sequence order, which attention output is invariant to.  K/V are computed for
the full 2048-row sequence on both cores of a batch pair (duplicated compute,
no collectives needed).

Per-core kernel (Tile):
  1. PE-transpose z -> zT (din-major), fp32.
  2. Q^T/K^T (d-major) and V (natural, with a ones column appended per head)
     projections via float32r matmuls; K^T/Q^T spilled to DRAM scratch.
  3. Per head: scores S^T = K Q^T (f32r), exp(s/8) on ACT -> bf16,
     P^T@V via matmul with V|ones (denominator accumulates in row 64),
     reciprocal + gpsimd partition-broadcast, normalized attn^T in fp32.
  4. Final projection attn @ w_o + b_o in f32r, bias via partition-broadcast.
"""

import os
import sys

_TRN_REPO = "/opt/trn_rl_repo"
if os.path.isdir(_TRN_REPO) and _TRN_REPO not in sys.path:
    sys.path.insert(0, _TRN_REPO)

import numpy as np

import concourse.bass as bass  # noqa: E402
import concourse.mybir as mybir  # noqa: E402
from concourse import bacc  # noqa: E402
from concourse.bass_utils import run_bass_kernel_spmd  # noqa: E402
from concourse.masks import make_identity  # noqa: E402
from concourse.tile import TileContext  # noqa: E402

F32 = mybir.dt.float32
F32R = mybir.dt.float32r
BF16 = mybir.dt.bfloat16
MULT = mybir.AluOpType.mult
ADD = mybir.AluOpType.add
EXP = mybir.ActivationFunctionType.Exp

N_CORES = 8
B, N, D = 4, 2048, 1024
H, HD = 16, 64
NQ = N // 2  # query rows per core
P = 128
DC = D // P  # 8 din/dout chunks of 128
NKC = N // P  # 16 key chunks of 128
SCALE = 1.0 / 8.0  # 1/sqrt(HD)


def _build():
    nc = bacc.Bacc("TRN2", target_bir_lowering=False, debug=False,
                   num_devices=N_CORES)
    z_d = nc.declare_dram_parameter("z", [N, D], F32, isOutput=False)
    wq_d = nc.declare_dram_parameter("w_q", [D, D], F32R, isOutput=False)
    wk_d = nc.declare_dram_parameter("w_k", [D, D], F32R, isOutput=False)
    wv_d = nc.declare_dram_parameter("w_v", [D, D], F32R, isOutput=False)
    wo_d = nc.declare_dram_parameter("w_o", [D, D], F32R, isOutput=False)
    bo_d = nc.declare_dram_parameter("b_o", [D], F32, isOutput=False)
    out_d = nc.declare_dram_parameter("out", [NQ, D], F32, isOutput=True)

    # DRAM scratch: K^T/Q^T in partition-major layout for clean reload.
    kts_d = nc.dram_tensor("kts", [P, DC, N], BF16)
    qts_d = nc.dram_tensor("qts", [P, DC, NQ], BF16)

    with TileContext(nc) as tc:
        with tc.tile_pool(name="const", bufs=1) as constp, \
             tc.tile_pool(name="vpool", bufs=1) as vpool:
            ident = constp.tile([P, P], F32)
            make_identity(nc, ident)
            # V' = [V_h | 1] per head: [P, key-chunk, head, 65] bf16
            vp = vpool.tile([P, NKC, H, HD + 1], BF16)
            nc.vector.memset(vp[:, :, :, HD], 1.0)
            # K^T/Q^T zero-padded scores operands live OUTSIDE the phase
            # pools so their zero rows are written at t=0 and phase-2 has no
            # SBUF zone handoff before the first scores matmul.
            ktz = vpool.tile([P, 2, N], BF16)
            qtz = vpool.tile([P, 2, NQ], BF16)
            nc.vector.memset(ktz[64:P, :, :], 0.0)
            nc.vector.memset(qtz[64:P, :, :], 0.0)

            # ---------------- Phase 1: zT + projections ----------------
            with tc.tile_pool(name="zin", bufs=1) as zinp, \
                 tc.tile_pool(name="zt", bufs=2) as ztp, \
                 tc.tile_pool(name="wt", bufs=3) as wtp, \
                 tc.tile_pool(name="stg", bufs=3) as stgp, \
                 tc.tile_pool(name="pst", bufs=2, space="PSUM") as pst, \
                 tc.tile_pool(name="psp", bufs=6, space="PSUM") as psp:

                zt_first = zinp.tile([P, 4, D], F32, name="zt_in")
                nc.sync.dma_start(
                    zt_first[:],
                    z_d[0:512, :].rearrange("(r p) d -> p r d", p=P))
                wk_sb = wtp.tile([P, DC, D], F32R, tag="w")
                nc.scalar.dma_start(wk_sb[:], wk_d.rearrange("(c p) o -> p c o", p=P))
                wq_sb = wtp.tile([P, DC, D], F32R, tag="w")
                nc.scalar.dma_start(wq_sb[:], wq_d.rearrange("(c p) o -> p c o", p=P))
                wv_sb = wtp.tile([P, DC, D], F32R, tag="w")
                nc.scalar.dma_start(wv_sb[:], wv_d.rearrange("(c p) o -> p c o", p=P))

                for n5 in range(N // 512):  # 4 big chunks of 512 seq rows
                    # transpose 512 z rows -> ztc [P, DC, 512]
                    ztc = ztp.tile([P, DC, 512], F32R)
                    if n5 == 0:
                        zt_in = zt_first
                    else:
                        zt_in = zinp.tile([P, 4, D], F32, name="zt_in")
                        nc.sync.dma_start(
                            zt_in[:],
                            z_d[n5 * 512:(n5 + 1) * 512, :].rearrange(
                                "(r p) d -> p r d", p=P))
                    for dc in range(DC):
                        ps = pst.tile([P, 512], F32)
                        for r in range(4):
                            nc.tensor.transpose(
                                ps[:, r * P:(r + 1) * P],
                                zt_in[:, r, dc * P:(dc + 1) * P],
                                ident[:])
                        nc.vector.tensor_copy(ztc[:, dc, :], ps[:])

                    # K^T chunk: [dout, 512] for all 8 dout chunks
                    for og in range(2):
                        pss = [psp.tile([P, 512], F32, name="pp") for _ in range(4)]
                        for dc in range(DC):
                            for j in range(4):
                                oc = og * 4 + j
                                nc.tensor.matmul(
                                    pss[j][:],
                                    lhsT=(wk_sb[:, dc, oc * P:(oc + 1) * P]),
                                    rhs=(ztc[:, dc, :]),
                                    start=(dc == 0), stop=(dc == DC - 1))
                        for j in range(4):
                            st = stgp.tile([P, 512], BF16)
                            nc.vector.tensor_copy(st[:], pss[j][:])
                            nc.scalar.dma_start(
                                kts_d[:, og * 4 + j, n5 * 512:(n5 + 1) * 512], st[:])

                    # Q^T chunk (first 1024 rows only)
                    if n5 < NQ // 512:
                        for og in range(2):
                            pss = [psp.tile([P, 512], F32, name="pp") for _ in range(4)]
                            for dc in range(DC):
                                for j in range(4):
                                    oc = og * 4 + j
                                    nc.tensor.matmul(
                                        pss[j][:],
                                        lhsT=(wq_sb[:, dc, oc * P:(oc + 1) * P]),
                                        rhs=(ztc[:, dc, :]),
                                        start=(dc == 0), stop=(dc == DC - 1))
                            for j in range(4):
                                st = stgp.tile([P, 512], BF16)
                                nc.vector.tensor_copy(st[:], pss[j][:])
                                nc.scalar.dma_start(
                                    qts_d[:, og * 4 + j, n5 * 512:(n5 + 1) * 512],
                                    st[:])

                    # V chunk: natural [k, dout] -> V' (strided per head)
                    for kcp in range(2):
                        pss = [psp.tile([P, 512], F32, name="pp") for _ in range(4)]
                        for dc in range(DC):
                            for i2 in range(2):
                                kc4 = kcp * 2 + i2
                                lh = (ztc[:, dc, kc4 * P:(kc4 + 1) * P])
                                for oc2 in range(2):
                                    nc.tensor.matmul(
                                        pss[i2 * 2 + oc2][:],
                                        lhsT=lh,
                                        rhs=(wv_sb[:, dc, oc2 * 512:(oc2 + 1) * 512]),
                                        start=(dc == 0), stop=(dc == DC - 1))
                        for i2 in range(2):
                            kcg = n5 * 4 + kcp * 2 + i2
                            for oc2 in range(2):
                                nc.vector.tensor_copy(
                                    vp[:, kcg, oc2 * 8:(oc2 + 1) * 8, 0:HD],
                                    pss[i2 * 2 + oc2].rearrange(
                                        "p (h d) -> p h d", d=HD))

            # ---------------- Phases 2+3 ----------------
            with tc.tile_pool(name="at", bufs=1) as atp:
                attnT = atp.tile([P, DC, NQ], BF16)
                bo_sb = atp.tile([1, D], F32)
                nc.scalar.dma_start(bo_sb[:], bo_d[None, :])
                bo_bc = atp.tile([P, D], F32)
                nc.gpsimd.partition_broadcast(bo_bc[:], bo_sb[:])
                wo_sb = atp.tile([P, DC, D], F32R)
                nc.scalar.dma_start(wo_sb[:], wo_d.rearrange("(c p) o -> p c o", p=P))
                wo16 = atp.tile([P, DC, D], BF16)
                nc.vector.tensor_copy(wo16[:], wo_sb[:])

                # Phase 2: attention per head
                # K^T/Q^T zero-padded to 128 contraction rows (rows 64-127 = 0)
                # so scores matmuls use the full PE array (keeps HAM warm).
                with tc.tile_pool(name="es", bufs=8) as esp, \
                     tc.tile_pool(name="rc", bufs=4) as recp, \
                     tc.tile_pool(name="pss", bufs=2, space="PSUM") as ssp, \
                     tc.tile_pool(name="pvo", bufs=4, space="PSUM") as pvp:
                    for h in range(H):
                        bf = h % 2
                        po = 64 * (h % 2)
                        nc.sync.dma_start(ktz[0:64, bf, :],
                                          kts_d[po:po + 64, h // 2, :])
                        nc.sync.dma_start(qtz[0:64, bf, :],
                                          qts_d[po:po + 64, h // 2, :])
                        pso = [pvp.tile([P, 512], F32, name="pvo") for _ in range(2)]
                        for kc in range(NKC):
                            ps = ssp.tile([P, NQ], F32, name="pss")
                            es = esp.tile([P, NQ], BF16)
                            for qc in range(2):
                                nc.tensor.matmul(
                                    ps[:, qc * 512:(qc + 1) * 512],
                                    lhsT=ktz[:, bf, kc * P:(kc + 1) * P],
                                    rhs=qtz[:, bf, qc * 512:(qc + 1) * 512])
                            nc.scalar.activation(es[:], ps[:], EXP, scale=SCALE)
                            lh = vp[:, kc, h, :]
                            for qc in range(2):
                                nc.tensor.matmul(
                                    pso[qc][0:HD + 1, :],
                                    lhsT=lh,
                                    rhs=es[:, qc * 512:(qc + 1) * 512],
                                    start=(kc == 0), stop=(kc == NKC - 1))
                        for qc in range(2):
                            rec = recp.tile([1, 512], F32, tag="rec")
                            nc.vector.reciprocal(rec[:], pso[qc][HD:HD + 1, :])
                            rb = recp.tile([64, 512], F32, tag="rb")
                            nc.gpsimd.partition_broadcast(rb[:], rec[:])
                            nc.vector.tensor_tensor(
                                attnT[po:po + 64, h // 2, qc * 512:(qc + 1) * 512],
                                pso[qc][0:HD, :], rb[:], MULT)

                # Phase 3: final projection + bias
                with tc.tile_pool(name="ot", bufs=4) as outp, \
                     tc.tile_pool(name="psf", bufs=4, space="PSUM") as fpp:
                    for q8 in range(NQ // P):
                        psf = [fpp.tile([P, 512], F32, name="pf") for _ in range(2)]
                        for dc in range(DC):
                            lh = (attnT[:, dc, q8 * P:(q8 + 1) * P])
                            for oc2 in range(2):
                                nc.tensor.matmul(
                                    psf[oc2][:],
                                    lhsT=lh,
                                    rhs=wo16[:, dc, oc2 * 512:(oc2 + 1) * 512],
                                    start=(dc == 0), stop=(dc == DC - 1))
                        for oc2 in range(2):
                            ot = outp.tile([P, 512], F32)
                            nc.vector.tensor_tensor(
                                ot[:], psf[oc2][:],
                                bo_bc[:, oc2 * 512:(oc2 + 1) * 512], ADD)
                            nc.sync.dma_start(
                                out_d[q8 * P:(q8 + 1) * P,
                                      oc2 * 512:(oc2 + 1) * 512], ot[:])

    nc.compile()
    return nc


_NC_CACHE = None


def _get_nc():
    global _NC_CACHE
    if _NC_CACHE is None:
        _NC_CACHE = _build()
    return _NC_CACHE


def _run(z, w_q, w_k, w_v, w_o, b_o, **spmd_kwargs):
    z = np.ascontiguousarray(np.asarray(z, dtype=np.float32))
    w_q = np.ascontiguousarray(np.asarray(w_q, dtype=np.float32))
    w_k = np.ascontiguousarray(np.asarray(w_k, dtype=np.float32))
    w_v = np.ascontiguousarray(np.asarray(w_v, dtype=np.float32))
    w_o = np.ascontiguousarray(np.asarray(w_o, dtype=np.float32))
    b_o = np.ascontiguousarray(np.asarray(b_o, dtype=np.float32))
    assert z.shape == (B, N, D)

    if not spmd_kwargs.get("trace"):
        # A stray BASS_TRACE in the environment would route through the NTFF
        # hook (absent in this image) and crash; force the no-trace path.
        os.environ["BASS_NEVER_TRACE"] = "1"

    nc = _get_nc()
    in_maps = []
    for c in range(N_CORES):
        b = c // 2
        off = (c % 2) * NQ
        zc = np.ascontiguousarray(np.concatenate([z[b, off:], z[b, :off]], axis=0))
        in_maps.append({"z": zc, "w_q": w_q, "w_k": w_k, "w_v": w_v,
                        "w_o": w_o, "b_o": b_o})

    res = run_bass_kernel_spmd(nc, in_maps, core_ids=list(range(N_CORES)),
                               **spmd_kwargs)
    out = np.empty((B, N, D), dtype=np.float32)
    for c in range(N_CORES):
        b = c // 2
        off = (c % 2) * NQ
        out[b, off:off + NQ, :] = res.results[c]["out"]
    return out, res


def kernel(z, w_q, w_k, w_v, w_o, b_o):
    out, _ = _run(z, w_q, w_k, w_v, w_o, b_o)
    return out



# revision 11
# speedup vs baseline: 1.1870x; 1.1870x over previous
"""Multi-head self-attention (B=4, N=2048, D=1024, H=16) on 8 trn2 NeuronCores.

Sharding: 8 shards = (batch, query-half).  Core c handles batch c//2 and query
rows [(c%2)*1024, (c%2)*1024+1024).  Each core receives its batch's z with the
rows rolled so that its query rows come first; rolling permutes the key/value
sequence order, which attention output is invariant to.  K/V are computed for
the full 2048-row sequence on both cores of a batch pair (duplicated compute,
no collectives needed).

Per-core kernel (Tile), restructured for PE/ACT overlap:
  - Everything SBUF-resident in bf16 (no DRAM spill of K^T/Q^T).
  - Per head-pair pipeline: projections for pair p are interleaved with
    attention for pair p-1 so the PE never idles while ACT drains the exp
    stream.
  - Scores via 64-row lhsT slices of the pair's K^T (no zero padding);
    per-head psum scores tile [128, 1024] -> one ACT exp instr.
  - PV in natural orientation: lhsT = exp-scores [128 keys, 128 q] slices,
    rhs = [V_h | 1] (65 cols) -> psum [128 q, 65] accumulated over key
    chunks; col 64 is the softmax denominator.  This streams 65 columns per
    accumulation step instead of 1024, halving PE attention work.
  - Normalize with per-partition reciprocal scalars on DVE, final projection
    from a PE re-transpose of the normalized attention output.
"""

import os
import sys

_TRN_REPO = "/opt/trn_rl_repo"
if os.path.isdir(_TRN_REPO) and _TRN_REPO not in sys.path:
    sys.path.insert(0, _TRN_REPO)

import numpy as np

import concourse.bass as bass  # noqa: E402
import concourse.mybir as mybir  # noqa: E402
from concourse import bacc  # noqa: E402
from concourse.bass_utils import run_bass_kernel_spmd  # noqa: E402
from concourse.masks import make_identity  # noqa: E402
from concourse.tile import TileContext  # noqa: E402

F32 = mybir.dt.float32
BF16 = mybir.dt.bfloat16
MULT = mybir.AluOpType.mult
ADD = mybir.AluOpType.add
EXP = mybir.ActivationFunctionType.Exp

N_CORES = 8
B, N, D = 4, 2048, 1024
H, HD = 16, 64
NQ = N // 2  # query rows per core
P = 128
DC = D // P  # 8 din/dout chunks of 128
NKC = N // P  # 16 key chunks of 128
NP = H // 2  # 8 head pairs
SCALE = 1.0 / 8.0  # 1/sqrt(HD)


def _build():
    nc = bacc.Bacc("TRN2", target_bir_lowering=False, debug=False,
                   num_devices=N_CORES)
    z_d = nc.declare_dram_parameter("z", [N, D], F32, isOutput=False)
    wq_d = nc.declare_dram_parameter("w_q", [D, D], F32, isOutput=False)
    wk_d = nc.declare_dram_parameter("w_k", [D, D], F32, isOutput=False)
    wv_d = nc.declare_dram_parameter("w_v", [D, D], F32, isOutput=False)
    wo_d = nc.declare_dram_parameter("w_o", [D, D], F32, isOutput=False)
    bo_d = nc.declare_dram_parameter("b_o", [D], F32, isOutput=False)
    out_d = nc.declare_dram_parameter("out", [NQ, D], F32, isOutput=True)

    with TileContext(nc) as tc:
        with tc.tile_pool(name="const", bufs=1) as constp, \
             tc.tile_pool(name="pers", bufs=1) as persp:
            identf = constp.tile([P, P], F32, name="identf")
            make_identity(nc, identf)
            ident16 = constp.tile([P, P], BF16, name="ident16")
            make_identity(nc, ident16)

            bo_sb = constp.tile([1, D], F32, name="bo_sb")
            nc.sync.dma_start(bo_sb[:], bo_d[None, :])
            bias_bc = constp.tile([P, D], F32, name="bias_bc")
            nc.gpsimd.partition_broadcast(bias_bc[:], bo_sb[:])

            # attention output, natural ([q-part, qc, din]) and transposed
            attnN = persp.tile([P, NQ // P, D], BF16, name="attnN")
            wo16 = persp.tile([P, DC, D], BF16, name="wo16")

            with tc.tile_pool(name="zts", bufs=1) as ztsp, \
                 tc.tile_pool(name="wbf", bufs=1) as wbfp, \
                 tc.tile_pool(name="wstg", bufs=1) as wstgp, \
                 tc.tile_pool(name="zin", bufs=2) as zinp, \
                 tc.tile_pool(name="kt", bufs=4) as ktp, \
                 tc.tile_pool(name="qt", bufs=4) as qtp, \
                 tc.tile_pool(name="von", bufs=2) as vonp, \
                 tc.tile_pool(name="es", bufs=4) as esp, \
                 tc.tile_pool(name="rec", bufs=4) as recp, \
                 tc.tile_pool(name="psproj", bufs=2, space="PSUM") as projps, \
                 tc.tile_pool(name="psscore", bufs=2, space="PSUM") as scoreps, \
                 tc.tile_pool(name="pspv", bufs=2, space="PSUM") as pvps:

                zT = ztsp.tile([P, DC, N], BF16, name="zT")
                wk16 = wbfp.tile([P, DC, D], BF16, name="wk16")
                wq16 = wbfp.tile([P, DC, D], BF16, name="wq16")
                wv16 = wbfp.tile([P, DC, D], BF16, name="wv16")

                # ---- weight load + cast (stage reused sequentially) ----
                def load_weight(dst16, src_d, eng):
                    for half in range(2):
                        stg = wstgp.tile([P, DC // 2, D], F32, name="wstg",
                                         tag="wstg")
                        eng.dma_start(
                            stg[:],
                            src_d[half * 512:(half + 1) * 512, :].rearrange(
                                "(c p) o -> p c o", p=P))
                        nc.vector.tensor_copy(
                            dst16[:, half * 4:(half + 1) * 4, :], stg[:])

                load_weight(wk16, wk_d, nc.scalar)
                load_weight(wq16, wq_d, nc.scalar)
                load_weight(wv16, wv_d, nc.scalar)

                # ---- z load + transpose into zT (bf16) ----
                def emit_ztranspose():
                    for ch in range(16):  # 128-row chunks
                        zin = zinp.tile([P, D], F32, name="zin", tag="zin")
                        nc.sync.dma_start(zin[:], z_d[ch * P:(ch + 1) * P, :])
                        for dg in range(2):
                            ps = projps.tile([P, 4, P], F32, name="zps",
                                             tag="pp")
                            for d4 in range(4):
                                dc = dg * 4 + d4
                                nc.tensor.transpose(
                                    ps[:, d4, :],
                                    zin[:, dc * P:(dc + 1) * P],
                                    identf[:])
                            nc.vector.tensor_copy(
                                zT[:, dg * 4:(dg + 1) * 4,
                                   ch * P:(ch + 1) * P],
                                ps[:])

                emit_ztranspose()

                # ---------- per-pair unit emitters ----------
                def make_proj_units(p, state):
                    """Projection chains for pair p (emitted lazily).

                    K^T/Q^T are stored per head, zero-padded to 128
                    contraction rows (head 0 occupies partitions 0:64 with
                    zeros at 64:128, head 1 the reverse) so scores matmuls
                    drive the full PE array and drains stay lane-aligned.
                    """
                    units = []
                    kTh = [ktp.tile([P, N], BF16, name="kTh", tag="kt")
                           for _ in range(2)]
                    qTh = [qtp.tile([P, NQ], BF16, name="qTh", tag="qt")
                           for _ in range(2)]
                    vONp = vonp.tile([P, NKC, 2, HD + 1], BF16, name="vONp",
                                     tag="von")
                    state[p] = (kTh, qTh, vONp)

                    def ones_unit():
                        nc.gpsimd.memset(vONp[:, :, :, HD], 1.0)
                        nc.gpsimd.memset(kTh[0][HD:P, :], 0.0)
                        nc.gpsimd.memset(kTh[1][0:HD, :], 0.0)
                        nc.gpsimd.memset(qTh[0][HD:P, :], 0.0)
                        nc.gpsimd.memset(qTh[1][0:HD, :], 0.0)
                    units.append(ones_unit)

                    def k_chain(s5):
                        def f():
                            ps = projps.tile([P, 512], F32, name="kps",
                                             tag="pp")
                            for dc in range(DC):
                                nc.tensor.matmul(
                                    ps[:],
                                    lhsT=wk16[:, dc, p * P:(p + 1) * P],
                                    rhs=zT[:, dc, s5 * 512:(s5 + 1) * 512],
                                    start=(dc == 0), stop=(dc == DC - 1))
                            nc.vector.tensor_copy(
                                kTh[0][0:HD, s5 * 512:(s5 + 1) * 512],
                                ps[0:HD, :])
                            nc.vector.tensor_copy(
                                kTh[1][HD:P, s5 * 512:(s5 + 1) * 512],
                                ps[HD:P, :])
                        return f

                    def q_chain(s5):
                        def f():
                            ps = projps.tile([P, 512], F32, name="qps",
                                             tag="pp")
                            for dc in range(DC):
                                nc.tensor.matmul(
                                    ps[:],
                                    lhsT=wq16[:, dc, p * P:(p + 1) * P],
                                    rhs=zT[:, dc, s5 * 512:(s5 + 1) * 512],
                                    start=(dc == 0), stop=(dc == DC - 1))
                            nc.vector.tensor_copy(
                                qTh[0][0:HD, s5 * 512:(s5 + 1) * 512],
                                ps[0:HD, :])
                            nc.vector.tensor_copy(
                                qTh[1][HD:P, s5 * 512:(s5 + 1) * 512],
                                ps[HD:P, :])
                        return f

                    def v_group(g):
                        def f():
                            vt = projps.tile([P, 4, P], F32, name="vps",
                                             tag="pp")
                            for kc4 in range(4):
                                kc = g * 4 + kc4
                                for dc in range(DC):
                                    nc.tensor.matmul(
                                        vt[:, kc4, :],
                                        lhsT=zT[:, dc, kc * P:(kc + 1) * P],
                                        rhs=wv16[:, dc, p * P:(p + 1) * P],
                                        start=(dc == 0), stop=(dc == DC - 1))
                            nc.vector.tensor_copy(
                                vONp[:, g * 4:(g + 1) * 4, :, 0:HD],
                                vt.rearrange("p k (h d) -> p k h d", d=HD))
                        return f

                    for s5 in range(4):
                        units.append(k_chain(s5))
                    for s5 in range(2):
                        units.append(q_chain(s5))
                    for g in range(4):
                        units.append(v_group(g))
                    return units

                def make_attn_units(p, state):
                    """Attention stream for pair p: per head 16 x (S, exp,
                    PV) + normalize."""
                    kTh, qTh, vONp = state[p]
                    units = []
                    for hh in range(2):
                        kTp, qTp = kTh[hh], qTh[hh]
                        pv_tiles = []

                        def head_setup(pv_tiles=pv_tiles):
                            for _ in range(2):
                                pv_tiles.append(pvps.tile(
                                    [P, 4, HD + 1], F32, name="pv", tag="pv"))

                        def kc_unit(kc, hh=hh, kTp=kTp, qTp=qTp,
                                    pv_tiles=pv_tiles):
                            def f():
                                ps = scoreps.tile([P, NQ], F32, name="sps",
                                                  tag="sc")
                                for qc2 in range(2):
                                    nc.tensor.matmul(
                                        ps[:, qc2 * 512:(qc2 + 1) * 512],
                                        lhsT=kTp[:, kc * P:(kc + 1) * P],
                                        rhs=qTp[:,
                                                qc2 * 512:(qc2 + 1) * 512])
                                es = esp.tile([P, NQ], BF16, name="es",
                                              tag="es")
                                nc.scalar.activation(es[:], ps[:], EXP,
                                                     scale=SCALE)
                                for qc in range(8):
                                    # one accumulation buffer per psum bank:
                                    # only the bank's first chain may zero it
                                    # (start); siblings accumulate onto the
                                    # zeroed buffer.
                                    nc.tensor.matmul(
                                        pv_tiles[qc // 4][:, qc % 4, :],
                                        lhsT=es[:, qc * P:(qc + 1) * P],
                                        rhs=vONp[:, kc, hh, :],
                                        start=(kc == 0 and qc % 4 == 0),
                                        stop=(kc == NKC - 1),
                                        skip_group_check=True)
                            return f

                        def norm_unit(half, p=p, hh=hh, pv_tiles=pv_tiles):
                            def f():
                                pv = pv_tiles[half]
                                rec = recp.tile([P, 4, 1], F32, name="rec",
                                                tag="rec")
                                nc.vector.reciprocal(
                                    rec[:], pv[:, :, HD:HD + 1])
                                for qc4 in range(4):
                                    qc = half * 4 + qc4
                                    nc.vector.tensor_scalar(
                                        attnN[:, qc,
                                              (2 * p + hh) * HD:
                                              (2 * p + hh + 1) * HD],
                                        pv[:, qc4, 0:HD],
                                        rec[:, qc4, :], None, MULT)
                            return f

                        units.append(head_setup)
                        for kc in range(NKC):
                            units.append(kc_unit(kc))
                        units.append(norm_unit(0))
                        units.append(norm_unit(1))
                    return units

                # ---------- pipelined emission ----------
                state = {}
                for p in range(NP + 1):
                    proj_units = make_proj_units(p, state) if p < NP else []
                    attn_units = make_attn_units(p - 1, state) if p > 0 else []
                    if not attn_units:
                        for u in proj_units:
                            u()
                        continue
                    # interleave: sprinkle proj units evenly through the
                    # (longer) attention stream
                    na, npj = len(attn_units), len(proj_units)
                    pi = 0
                    for i, u in enumerate(attn_units):
                        u()
                        want = ((i + 1) * npj) // na
                        while pi < want:
                            proj_units[pi]()
                            pi += 1
                    while pi < npj:
                        proj_units[pi]()
                        pi += 1
                    if p == 5:
                        # preload + cast w_o while attention still runs
                        load_weight(wo16, wo_d, nc.sync)

            # ---------------- tail: transpose + final projection ----------
            with tc.tile_pool(name="at", bufs=1) as atp, \
                 tc.tile_pool(name="ot", bufs=4) as outp, \
                 tc.tile_pool(name="pstp", bufs=2, space="PSUM") as tpps, \
                 tc.tile_pool(name="psf", bufs=2, space="PSUM") as fpp:
                attnT = atp.tile([P, DC, NQ], BF16, name="attnT")
                for qc in range(NQ // P):
                    for dg in range(2):
                        tp = tpps.tile([P, 4, P], BF16, name="tp", tag="tp")
                        for d4 in range(4):
                            dinc = dg * 4 + d4
                            nc.tensor.transpose(
                                tp[:, d4, :],
                                attnN[:, qc, dinc * P:(dinc + 1) * P],
                                ident16[:])
                        nc.vector.tensor_copy(
                            attnT[:, dg * 4:(dg + 1) * 4,
                                  qc * P:(qc + 1) * P],
                            tp[:])
                for qc in range(NQ // P):
                    for oc2 in range(2):
                        po = fpp.tile([P, 512], F32, name="po", tag="po")
                        for dc in range(DC):
                            nc.tensor.matmul(
                                po[:],
                                lhsT=attnT[:, dc, qc * P:(qc + 1) * P],
                                rhs=wo16[:, dc, oc2 * 512:(oc2 + 1) * 512],
                                start=(dc == 0), stop=(dc == DC - 1))
                        ot = outp.tile([P, 512], F32, name="ot", tag="ot")
                        nc.vector.tensor_tensor(
                            ot[:], po[:], bias_bc[:, oc2 * 512:(oc2 + 1) * 512],
                            ADD)
                        nc.sync.dma_start(
                            out_d[qc * P:(qc + 1) * P,
                                  oc2 * 512:(oc2 + 1) * 512], ot[:])

    nc.compile()
    return nc


_NC_CACHE = None


def _get_nc():
    global _NC_CACHE
    if _NC_CACHE is None:
        _NC_CACHE = _build()
    return _NC_CACHE


def _run(z, w_q, w_k, w_v, w_o, b_o, **spmd_kwargs):
    z = np.ascontiguousarray(np.asarray(z, dtype=np.float32))
    w_q = np.ascontiguousarray(np.asarray(w_q, dtype=np.float32))
    w_k = np.ascontiguousarray(np.asarray(w_k, dtype=np.float32))
    w_v = np.ascontiguousarray(np.asarray(w_v, dtype=np.float32))
    w_o = np.ascontiguousarray(np.asarray(w_o, dtype=np.float32))
    b_o = np.ascontiguousarray(np.asarray(b_o, dtype=np.float32))
    assert z.shape == (B, N, D)

    if not spmd_kwargs.get("trace"):
        # A stray BASS_TRACE in the environment would route through the NTFF
        # hook (absent in this image) and crash; force the no-trace path.
        os.environ["BASS_NEVER_TRACE"] = "1"

    nc = _get_nc()
    in_maps = []
    for c in range(N_CORES):
        b = c // 2
        off = (c % 2) * NQ
        zc = np.ascontiguousarray(np.concatenate([z[b, off:], z[b, :off]], axis=0))
        in_maps.append({"z": zc, "w_q": w_q, "w_k": w_k, "w_v": w_v,
                        "w_o": w_o, "b_o": b_o})

    res = run_bass_kernel_spmd(nc, in_maps, core_ids=list(range(N_CORES)),
                               **spmd_kwargs)
    out = np.empty((B, N, D), dtype=np.float32)
    for c in range(N_CORES):
        b = c // 2
        off = (c % 2) * NQ
        out[b, off:off + NQ, :] = res.results[c]["out"]
    return out, res


def kernel(z, w_q, w_k, w_v, w_o, b_o):
    out, _ = _run(z, w_q, w_k, w_v, w_o, b_o)
    return out


# revision 14
# speedup vs baseline: 1.2346x; 1.0401x over previous
"""Multi-head self-attention (B=4, N=2048, D=1024, H=16) on 8 trn2 NeuronCores.

Sharding: 8 shards = (batch, query-half).  Core c handles batch c//2 and query
rows [(c%2)*1024, (c%2)*1024+1024).  Each core receives its batch's z with the
rows rolled so that its query rows come first; rolling permutes the key/value
sequence order, which attention output is invariant to.  K/V are computed for
the full 2048-row sequence on both cores of a batch pair (duplicated compute,
no collectives needed).

Per-core kernel (Tile), restructured for PE/ACT overlap:
  - Everything SBUF-resident in bf16 (no DRAM spill of K^T/Q^T).
  - Per head-pair pipeline: projections for pair p are interleaved with
    attention for pair p-1 so the PE never idles while ACT drains the exp
    stream.
  - Scores via 64-row lhsT slices of the pair's K^T (no zero padding);
    per-head psum scores tile [128, 1024] -> one ACT exp instr.
  - PV in natural orientation: lhsT = exp-scores [128 keys, 128 q] slices,
    rhs = [V_h | 1] (65 cols) -> psum [128 q, 65] accumulated over key
    chunks; col 64 is the softmax denominator.  This streams 65 columns per
    accumulation step instead of 1024, halving PE attention work.
  - Normalize with per-partition reciprocal scalars on DVE, final projection
    from a PE re-transpose of the normalized attention output.
"""

import os
import sys

_TRN_REPO = "/opt/trn_rl_repo"
if os.path.isdir(_TRN_REPO) and _TRN_REPO not in sys.path:
    sys.path.insert(0, _TRN_REPO)

import numpy as np

import concourse.bass as bass  # noqa: E402
import concourse.mybir as mybir  # noqa: E402
from concourse import bacc  # noqa: E402
from concourse.bass_utils import run_bass_kernel_spmd  # noqa: E402
from concourse.masks import make_identity  # noqa: E402
from concourse.tile import TileContext  # noqa: E402

F32 = mybir.dt.float32
BF16 = mybir.dt.bfloat16
MULT = mybir.AluOpType.mult
ADD = mybir.AluOpType.add
EXP = mybir.ActivationFunctionType.Exp

N_CORES = 8
B, N, D = 4, 2048, 1024
H, HD = 16, 64
NQ = N // 2  # query rows per core
P = 128
DC = D // P  # 8 din/dout chunks of 128
NKC = N // P  # 16 key chunks of 128
NP = H // 2  # 8 head pairs
SCALE = 1.0 / 8.0  # 1/sqrt(HD)


def _build():
    nc = bacc.Bacc("TRN2", target_bir_lowering=False, debug=False,
                   num_devices=N_CORES)
    z_d = nc.declare_dram_parameter("z", [N, D], F32, isOutput=False)
    wq_d = nc.declare_dram_parameter("w_q", [D, D], F32, isOutput=False)
    wk_d = nc.declare_dram_parameter("w_k", [D, D], F32, isOutput=False)
    wv_d = nc.declare_dram_parameter("w_v", [D, D], F32, isOutput=False)
    wo_d = nc.declare_dram_parameter("w_o", [D, D], F32, isOutput=False)
    bo_d = nc.declare_dram_parameter("b_o", [D], F32, isOutput=False)
    out_d = nc.declare_dram_parameter("out", [NQ, D], F32, isOutput=True)

    with TileContext(nc) as tc:
        with tc.tile_pool(name="const", bufs=1) as constp, \
             tc.tile_pool(name="pers", bufs=1) as persp:
            identf = constp.tile([P, P], F32, name="identf")
            make_identity(nc, identf)
            ident16 = constp.tile([P, P], BF16, name="ident16")
            make_identity(nc, ident16)

            bo_sb = constp.tile([1, D], F32, name="bo_sb")
            nc.sync.dma_start(bo_sb[:], bo_d[None, :])
            bias_bc = constp.tile([P, D], F32, name="bias_bc")
            nc.gpsimd.partition_broadcast(bias_bc[:], bo_sb[:])

            # attention output, natural ([q-part, qc, din]) and transposed
            attnN = persp.tile([P, NQ // P, D], BF16, name="attnN")
            wo16 = persp.tile([P, DC, D], BF16, name="wo16")

            with tc.tile_pool(name="zts", bufs=1) as ztsp, \
                 tc.tile_pool(name="wpair", bufs=2) as wpairp, \
                 tc.tile_pool(name="wstg", bufs=2) as wstgp, \
                 tc.tile_pool(name="wostg", bufs=1) as wostgp, \
                 tc.tile_pool(name="zin", bufs=2) as zinp, \
                 tc.tile_pool(name="kqv", bufs=1) as kqvp, \
                 tc.tile_pool(name="es", bufs=6) as esp, \
                 tc.tile_pool(name="rec", bufs=4) as recp, \
                 tc.tile_pool(name="psproj", bufs=2, space="PSUM") as projps, \
                 tc.tile_pool(name="psscore", bufs=2, space="PSUM") as scoreps, \
                 tc.tile_pool(name="pspv", bufs=2, space="PSUM") as pvps:

                zT = ztsp.tile([P, DC, N], BF16, name="zT")

                # persistent, manually double-buffered K^T/Q^T/V' tiles.
                # K^T/Q^T are per head, zero-padded to 128 contraction rows
                # (head 0 at partitions 0:64, head 1 at 64:128); the pad and
                # the V' ones-column are memset ONCE here, off the critical
                # path, instead of per pair.
                kbuf = [[kqvp.tile([P, N], BF16, name="kb", tag=f"kb{i}{hh}")
                         for hh in range(2)] for i in range(2)]
                qbuf = [[kqvp.tile([P, NQ], BF16, name="qb", tag=f"qb{i}{hh}")
                         for hh in range(2)] for i in range(2)]
                vbuf = [kqvp.tile([P, NKC, 2, HD + 1], BF16, name="vb",
                                  tag=f"vb{i}") for i in range(2)]
                for i in range(2):
                    nc.gpsimd.memset(kbuf[i][0][HD:P, :], 0.0)
                    nc.gpsimd.memset(kbuf[i][1][0:HD, :], 0.0)
                    nc.gpsimd.memset(qbuf[i][0][HD:P, :], 0.0)
                    nc.gpsimd.memset(qbuf[i][1][0:HD, :], 0.0)
                    nc.gpsimd.memset(vbuf[i][:, :, :, HD], 1.0)

                # ---- z load + transpose into zT (bf16) ----
                def emit_ztranspose():
                    for ch in range(16):  # 128-row chunks
                        zin = zinp.tile([P, D], F32, name="zin", tag="zin")
                        nc.sync.dma_start(zin[:], z_d[ch * P:(ch + 1) * P, :])
                        for dg in range(2):
                            ps = projps.tile([P, 4, P], F32, name="zps",
                                             tag="pp")
                            for d4 in range(4):
                                dc = dg * 4 + d4
                                nc.tensor.transpose(
                                    ps[:, d4, :],
                                    zin[:, dc * P:(dc + 1) * P],
                                    identf[:])
                            nc.vector.tensor_copy(
                                zT[:, dg * 4:(dg + 1) * 4,
                                   ch * P:(ch + 1) * P],
                                ps[:])

                emit_ztranspose()

                # ---------- per-pair unit emitters ----------
                def make_proj_units(p, state):
                    """Projection chains for pair p (emitted lazily), fed by
                    per-pair weight column slices so pair 0 starts early."""
                    units = []
                    kTh, qTh = kbuf[p % 2], qbuf[p % 2]
                    vONp = vbuf[p % 2]
                    state[p] = (kTh, qTh, vONp)
                    w16 = {}

                    def w_load(nm, src_d):
                        def f():
                            stg = wstgp.tile([P, DC, P], F32, name="wstg",
                                             tag="wstg")
                            nc.sync.dma_start(
                                stg[:],
                                src_d[:, p * P:(p + 1) * P].rearrange(
                                    "(c p) o -> p c o", p=P))
                            w16[nm] = wpairp.tile([P, DC, P], BF16,
                                                  name="w16", tag=f"w{nm}")
                            nc.vector.tensor_copy(w16[nm][:], stg[:])
                        return f

                    def k_chain(s5):
                        def f():
                            ps = projps.tile([P, 512], F32, name="kps",
                                             tag="pp")
                            for dc in range(DC):
                                nc.tensor.matmul(
                                    ps[:],
                                    lhsT=w16["k"][:, dc, :],
                                    rhs=zT[:, dc, s5 * 512:(s5 + 1) * 512],
                                    start=(dc == 0), stop=(dc == DC - 1))
                            nc.vector.tensor_copy(
                                kTh[0][0:HD, s5 * 512:(s5 + 1) * 512],
                                ps[0:HD, :])
                            nc.vector.tensor_copy(
                                kTh[1][HD:P, s5 * 512:(s5 + 1) * 512],
                                ps[HD:P, :])
                        return f

                    def q_chain(s5):
                        def f():
                            ps = projps.tile([P, 512], F32, name="qps",
                                             tag="pp")
                            for dc in range(DC):
                                nc.tensor.matmul(
                                    ps[:],
                                    lhsT=w16["q"][:, dc, :],
                                    rhs=zT[:, dc, s5 * 512:(s5 + 1) * 512],
                                    start=(dc == 0), stop=(dc == DC - 1))
                            nc.vector.tensor_copy(
                                qTh[0][0:HD, s5 * 512:(s5 + 1) * 512],
                                ps[0:HD, :])
                            nc.vector.tensor_copy(
                                qTh[1][HD:P, s5 * 512:(s5 + 1) * 512],
                                ps[HD:P, :])
                        return f

                    def v_group(g):
                        def f():
                            vt = projps.tile([P, 4, P], F32, name="vps",
                                             tag="pp")
                            for kc4 in range(4):
                                kc = g * 4 + kc4
                                for dc in range(DC):
                                    nc.tensor.matmul(
                                        vt[:, kc4, :],
                                        lhsT=zT[:, dc, kc * P:(kc + 1) * P],
                                        rhs=w16["v"][:, dc, :],
                                        start=(dc == 0), stop=(dc == DC - 1))
                            nc.vector.tensor_copy(
                                vONp[:, g * 4:(g + 1) * 4, :, 0:HD],
                                vt.rearrange("p k (h d) -> p k h d", d=HD))
                        return f

                    units.append(w_load("k", wk_d))
                    units.append(w_load("q", wq_d))
                    units.append(w_load("v", wv_d))
                    for s5 in range(4):
                        units.append(k_chain(s5))
                    for s5 in range(2):
                        units.append(q_chain(s5))
                    for g in range(4):
                        units.append(v_group(g))
                    return units

                def make_attn_units(p, state):
                    """Attention stream for pair p: per head 16 x (S, exp,
                    PV) + normalize."""
                    kTh, qTh, vONp = state[p]
                    units = []
                    for hh in range(2):
                        kTp, qTp = kTh[hh], qTh[hh]
                        pv_tiles = []

                        def head_setup(pv_tiles=pv_tiles):
                            for _ in range(2):
                                pv_tiles.append(pvps.tile(
                                    [P, 4, HD + 1], F32, name="pv", tag="pv"))

                        def kc_unit(kc, hh=hh, kTp=kTp, qTp=qTp,
                                    pv_tiles=pv_tiles):
                            def f():
                                ps = scoreps.tile([P, NQ], F32, name="sps",
                                                  tag="sc")
                                for qc2 in range(2):
                                    nc.tensor.matmul(
                                        ps[:, qc2 * 512:(qc2 + 1) * 512],
                                        lhsT=kTp[:, kc * P:(kc + 1) * P],
                                        rhs=qTp[:,
                                                qc2 * 512:(qc2 + 1) * 512])
                                es = esp.tile([P, NQ], BF16, name="es",
                                              tag="es")
                                nc.scalar.activation(es[:], ps[:], EXP,
                                                     scale=SCALE)
                                for qc in range(8):
                                    # one accumulation buffer per psum bank:
                                    # only the bank's first chain may zero it
                                    # (start); siblings accumulate onto the
                                    # zeroed buffer.
                                    nc.tensor.matmul(
                                        pv_tiles[qc // 4][:, qc % 4, :],
                                        lhsT=es[:, qc * P:(qc + 1) * P],
                                        rhs=vONp[:, kc, hh, :],
                                        start=(kc == 0 and qc % 4 == 0),
                                        stop=(kc == NKC - 1),
                                        skip_group_check=True)
                            return f

                        def norm_unit(half, p=p, hh=hh, pv_tiles=pv_tiles):
                            def f():
                                pv = pv_tiles[half]
                                rec = recp.tile([P, 4, 1], F32, name="rec",
                                                tag="rec")
                                nc.vector.reciprocal(
                                    rec[:], pv[:, :, HD:HD + 1])
                                for qc4 in range(4):
                                    qc = half * 4 + qc4
                                    nc.vector.tensor_scalar(
                                        attnN[:, qc,
                                              (2 * p + hh) * HD:
                                              (2 * p + hh + 1) * HD],
                                        pv[:, qc4, 0:HD],
                                        rec[:, qc4, :], None, MULT)
                            return f

                        units.append(head_setup)
                        for kc in range(NKC):
                            units.append(kc_unit(kc))
                        units.append(norm_unit(0))
                        units.append(norm_unit(1))
                    return units

                # ---------- pipelined emission ----------
                state = {}
                for p in range(NP + 1):
                    proj_units = make_proj_units(p, state) if p < NP else []
                    attn_units = make_attn_units(p - 1, state) if p > 0 else []
                    if not attn_units:
                        for u in proj_units:
                            u()
                        continue
                    # interleave: sprinkle proj units evenly through the
                    # (longer) attention stream
                    na, npj = len(attn_units), len(proj_units)
                    pi = 0
                    for i, u in enumerate(attn_units):
                        u()
                        want = ((i + 1) * npj) // na
                        while pi < want:
                            proj_units[pi]()
                            pi += 1
                    while pi < npj:
                        proj_units[pi]()
                        pi += 1
                    if p == 5:
                        # preload + cast w_o while attention still runs
                        for half in range(2):
                            stg = wostgp.tile([P, DC // 2, D], F32,
                                              name="wostg", tag="wostg")
                            nc.sync.dma_start(
                                stg[:],
                                wo_d[half * 512:(half + 1) * 512, :]
                                .rearrange("(c p) o -> p c o", p=P))
                            nc.vector.tensor_copy(
                                wo16[:, half * 4:(half + 1) * 4, :], stg[:])

            # ---------------- tail: transpose + final projection ----------
            with tc.tile_pool(name="at", bufs=1) as atp, \
                 tc.tile_pool(name="ot", bufs=4) as outp, \
                 tc.tile_pool(name="pstp", bufs=2, space="PSUM") as tpps, \
                 tc.tile_pool(name="psf", bufs=2, space="PSUM") as fpp:
                attnT = atp.tile([P, DC, NQ], BF16, name="attnT")
                for qc in range(NQ // P):
                    for dg in range(2):
                        tp = tpps.tile([P, 4, P], BF16, name="tp", tag="tp")
                        for d4 in range(4):
                            dinc = dg * 4 + d4
                            nc.tensor.transpose(
                                tp[:, d4, :],
                                attnN[:, qc, dinc * P:(dinc + 1) * P],
                                ident16[:])
                        nc.vector.tensor_copy(
                            attnT[:, dg * 4:(dg + 1) * 4,
                                  qc * P:(qc + 1) * P],
                            tp[:])
                for qc in range(NQ // P):
                    for oc2 in range(2):
                        po = fpp.tile([P, 512], F32, name="po", tag="po")
                        for dc in range(DC):
                            nc.tensor.matmul(
                                po[:],
                                lhsT=attnT[:, dc, qc * P:(qc + 1) * P],
                                rhs=wo16[:, dc, oc2 * 512:(oc2 + 1) * 512],
                                start=(dc == 0), stop=(dc == DC - 1))
                        ot = outp.tile([P, 512], F32, name="ot", tag="ot")
                        nc.vector.tensor_tensor(
                            ot[:], po[:], bias_bc[:, oc2 * 512:(oc2 + 1) * 512],
                            ADD)
                        nc.sync.dma_start(
                            out_d[qc * P:(qc + 1) * P,
                                  oc2 * 512:(oc2 + 1) * 512], ot[:])

    nc.compile()
    return nc


_NC_CACHE = None


def _get_nc():
    global _NC_CACHE
    if _NC_CACHE is None:
        _NC_CACHE = _build()
    return _NC_CACHE


def _run(z, w_q, w_k, w_v, w_o, b_o, **spmd_kwargs):
    z = np.ascontiguousarray(np.asarray(z, dtype=np.float32))
    w_q = np.ascontiguousarray(np.asarray(w_q, dtype=np.float32))
    w_k = np.ascontiguousarray(np.asarray(w_k, dtype=np.float32))
    w_v = np.ascontiguousarray(np.asarray(w_v, dtype=np.float32))
    w_o = np.ascontiguousarray(np.asarray(w_o, dtype=np.float32))
    b_o = np.ascontiguousarray(np.asarray(b_o, dtype=np.float32))
    assert z.shape == (B, N, D)

    if not spmd_kwargs.get("trace"):
        # A stray BASS_TRACE in the environment would route through the NTFF
        # hook (absent in this image) and crash; force the no-trace path.
        os.environ["BASS_NEVER_TRACE"] = "1"

    nc = _get_nc()
    in_maps = []
    for c in range(N_CORES):
        b = c // 2
        off = (c % 2) * NQ
        zc = np.ascontiguousarray(np.concatenate([z[b, off:], z[b, :off]], axis=0))
        in_maps.append({"z": zc, "w_q": w_q, "w_k": w_k, "w_v": w_v,
                        "w_o": w_o, "b_o": b_o})

    res = run_bass_kernel_spmd(nc, in_maps, core_ids=list(range(N_CORES)),
                               **spmd_kwargs)
    out = np.empty((B, N, D), dtype=np.float32)
    for c in range(N_CORES):
        b = c // 2
        off = (c % 2) * NQ
        out[b, off:off + NQ, :] = res.results[c]["out"]
    return out, res


def kernel(z, w_q, w_k, w_v, w_o, b_o):
    out, _ = _run(z, w_q, w_k, w_v, w_o, b_o)
    return out


# revision 18
# speedup vs baseline: 1.2364x; 1.0015x over previous
"""Multi-head self-attention (B=4, N=2048, D=1024, H=16) on 8 trn2 NeuronCores.

Sharding: 8 shards = (batch, query-half).  Core c handles batch c//2 and query
rows [(c%2)*1024, (c%2)*1024+1024).  Each core receives its batch's z with the
rows rolled so that its query rows come first; rolling permutes the key/value
sequence order, which attention output is invariant to.  K/V are computed for
the full 2048-row sequence on both cores of a batch pair (duplicated compute,
no collectives needed).

Per-core kernel (Tile), restructured for PE/ACT overlap:
  - Everything SBUF-resident in bf16 (no DRAM spill of K^T/Q^T).
  - Per head-pair pipeline: projections for pair p are interleaved with
    attention for pair p-1 so the PE never idles while ACT drains the exp
    stream.
  - Scores via 64-row lhsT slices of the pair's K^T (no zero padding);
    per-head psum scores tile [128, 1024] -> one ACT exp instr.
  - PV in natural orientation: lhsT = exp-scores [128 keys, 128 q] slices,
    rhs = [V_h | 1] (65 cols) -> psum [128 q, 65] accumulated over key
    chunks; col 64 is the softmax denominator.  This streams 65 columns per
    accumulation step instead of 1024, halving PE attention work.
  - Normalize with per-partition reciprocal scalars on DVE, final projection
    from a PE re-transpose of the normalized attention output.
"""

import os
import sys

_TRN_REPO = "/opt/trn_rl_repo"
if os.path.isdir(_TRN_REPO) and _TRN_REPO not in sys.path:
    sys.path.insert(0, _TRN_REPO)

import numpy as np

import concourse.bass as bass  # noqa: E402
import concourse.mybir as mybir  # noqa: E402
from concourse import bacc  # noqa: E402
from concourse.bass_utils import run_bass_kernel_spmd  # noqa: E402
from concourse.masks import make_identity  # noqa: E402
from concourse.tile import TileContext  # noqa: E402

F32 = mybir.dt.float32
BF16 = mybir.dt.bfloat16
MULT = mybir.AluOpType.mult
ADD = mybir.AluOpType.add
EXP = mybir.ActivationFunctionType.Exp

N_CORES = 8
B, N, D = 4, 2048, 1024
H, HD = 16, 64
NQ = N // 2  # query rows per core
P = 128
DC = D // P  # 8 din/dout chunks of 128
NKC = N // P  # 16 key chunks of 128
NP = H // 2  # 8 head pairs
SCALE = 1.0 / 8.0  # 1/sqrt(HD)


def _build():
    nc = bacc.Bacc("TRN2", target_bir_lowering=False, debug=False,
                   num_devices=N_CORES)
    z_d = nc.declare_dram_parameter("z", [N, D], F32, isOutput=False)
    wq_d = nc.declare_dram_parameter("w_q", [D, D], F32, isOutput=False)
    wk_d = nc.declare_dram_parameter("w_k", [D, D], F32, isOutput=False)
    wv_d = nc.declare_dram_parameter("w_v", [D, D], F32, isOutput=False)
    wo_d = nc.declare_dram_parameter("w_o", [D, D], F32, isOutput=False)
    bo_d = nc.declare_dram_parameter("b_o", [D], F32, isOutput=False)
    out_d = nc.declare_dram_parameter("out", [NQ, D], F32, isOutput=True)

    with TileContext(nc) as tc:
        with tc.tile_pool(name="const", bufs=1) as constp, \
             tc.tile_pool(name="pers", bufs=1) as persp:
            identf = constp.tile([P, P], F32, name="identf")
            make_identity(nc, identf)
            ident16 = constp.tile([P, P], BF16, name="ident16")
            make_identity(nc, ident16)

            bo_sb = constp.tile([1, D], F32, name="bo_sb")
            nc.sync.dma_start(bo_sb[:], bo_d[None, :])
            bias_bc = constp.tile([P, D], F32, name="bias_bc")
            nc.gpsimd.partition_broadcast(bias_bc[:], bo_sb[:])

            # attention output, natural ([q-part, qc, din]) and transposed
            attnN = persp.tile([P, NQ // P, D], BF16, name="attnN")
            attnT = persp.tile([P, DC, NQ], BF16, name="attnT")
            wo16 = persp.tile([P, DC, D], BF16, name="wo16")

            with tc.tile_pool(name="zts", bufs=1) as ztsp, \
                 tc.tile_pool(name="wpair", bufs=2) as wpairp, \
                 tc.tile_pool(name="wstg", bufs=2) as wstgp, \
                 tc.tile_pool(name="wostg", bufs=1) as wostgp, \
                 tc.tile_pool(name="zin", bufs=2) as zinp, \
                 tc.tile_pool(name="kqv", bufs=1) as kqvp, \
                 tc.tile_pool(name="es", bufs=6) as esp, \
                 tc.tile_pool(name="rec", bufs=4) as recp, \
                 tc.tile_pool(name="psproj", bufs=2, space="PSUM") as projps, \
                 tc.tile_pool(name="psscore", bufs=2, space="PSUM") as scoreps, \
                 tc.tile_pool(name="pspv", bufs=2, space="PSUM") as pvps:

                zT = ztsp.tile([P, DC, N], BF16, name="zT")

                # persistent, manually double-buffered K^T/Q^T/V' tiles.
                # K^T/Q^T are per head, zero-padded to 128 contraction rows
                # (head 0 at partitions 0:64, head 1 at 64:128); the pad and
                # the V' ones-column are memset ONCE here, off the critical
                # path, instead of per pair.
                kbuf = [[kqvp.tile([P, N], BF16, name="kb", tag=f"kb{i}{hh}")
                         for hh in range(2)] for i in range(2)]
                qbuf = [[kqvp.tile([P, NQ], BF16, name="qb", tag=f"qb{i}{hh}")
                         for hh in range(2)] for i in range(2)]
                vbuf = [kqvp.tile([P, NKC, 2, HD + 1], BF16, name="vb",
                                  tag=f"vb{i}") for i in range(2)]
                for i in range(2):
                    nc.gpsimd.memset(kbuf[i][0][HD:P, :], 0.0)
                    nc.gpsimd.memset(kbuf[i][1][0:HD, :], 0.0)
                    nc.gpsimd.memset(qbuf[i][0][HD:P, :], 0.0)
                    nc.gpsimd.memset(qbuf[i][1][0:HD, :], 0.0)
                    nc.gpsimd.memset(vbuf[i][:, :, :, HD], 1.0)

                # ---- z load + cast + bf16 transpose into zT ----
                def zt_unit(ch):
                    def f():
                        zin = zinp.tile([P, D], F32, name="zin", tag="zin")
                        nc.sync.dma_start(zin[:], z_d[ch * P:(ch + 1) * P, :])
                        zc = zinp.tile([P, D], BF16, name="zc", tag="zc")
                        nc.vector.tensor_copy(zc[:], zin[:])
                        for dg in range(2):
                            ps = projps.tile([P, 4, P], BF16, name="zps",
                                             tag="pp")
                            for d4 in range(4):
                                dc = dg * 4 + d4
                                nc.tensor.transpose(
                                    ps[:, d4, :],
                                    zc[:, dc * P:(dc + 1) * P],
                                    ident16[:])
                            nc.vector.tensor_copy(
                                zT[:, dg * 4:(dg + 1) * 4,
                                   ch * P:(ch + 1) * P],
                                ps[:])
                    return f

                # ---------- per-pair unit emitters ----------
                def make_proj_units(p, state):
                    """Projection chains for pair p (emitted lazily), fed by
                    per-pair weight column slices so pair 0 starts early."""
                    units = []
                    kTh, qTh = kbuf[p % 2], qbuf[p % 2]
                    vONp = vbuf[p % 2]
                    state[p] = (kTh, qTh, vONp)
                    w16 = {}

                    def w_load(nm, src_d):
                        def f():
                            stg = wstgp.tile([P, DC, P], F32, name="wstg",
                                             tag="wstg")
                            nc.sync.dma_start(
                                stg[:],
                                src_d[:, p * P:(p + 1) * P].rearrange(
                                    "(c p) o -> p c o", p=P))
                            w16[nm] = wpairp.tile([P, DC, P], BF16,
                                                  name="w16", tag=f"w{nm}")
                            nc.vector.tensor_copy(w16[nm][:], stg[:])
                        return f

                    def k_chain(s5):
                        def f():
                            ps = projps.tile([P, 512], F32, name="kps",
                                             tag="pp")
                            for dc in range(DC):
                                nc.tensor.matmul(
                                    ps[:],
                                    lhsT=w16["k"][:, dc, :],
                                    rhs=zT[:, dc, s5 * 512:(s5 + 1) * 512],
                                    start=(dc == 0), stop=(dc == DC - 1))
                            nc.vector.tensor_copy(
                                kTh[0][0:HD, s5 * 512:(s5 + 1) * 512],
                                ps[0:HD, :])
                            nc.vector.tensor_copy(
                                kTh[1][HD:P, s5 * 512:(s5 + 1) * 512],
                                ps[HD:P, :])
                        return f

                    def q_chain(s5):
                        def f():
                            ps = projps.tile([P, 512], F32, name="qps",
                                             tag="pp")
                            for dc in range(DC):
                                nc.tensor.matmul(
                                    ps[:],
                                    lhsT=w16["q"][:, dc, :],
                                    rhs=zT[:, dc, s5 * 512:(s5 + 1) * 512],
                                    start=(dc == 0), stop=(dc == DC - 1))
                            nc.vector.tensor_copy(
                                qTh[0][0:HD, s5 * 512:(s5 + 1) * 512],
                                ps[0:HD, :])
                            nc.vector.tensor_copy(
                                qTh[1][HD:P, s5 * 512:(s5 + 1) * 512],
                                ps[HD:P, :])
                        return f

                    def v_group(g):
                        def f():
                            vt = projps.tile([P, 4, P], F32, name="vps",
                                             tag="pp")
                            for kc4 in range(4):
                                kc = g * 4 + kc4
                                for dc in range(DC):
                                    nc.tensor.matmul(
                                        vt[:, kc4, :],
                                        lhsT=zT[:, dc, kc * P:(kc + 1) * P],
                                        rhs=w16["v"][:, dc, :],
                                        start=(dc == 0), stop=(dc == DC - 1))
                            nc.vector.tensor_copy(
                                vONp[:, g * 4:(g + 1) * 4, :, 0:HD],
                                vt.rearrange("p k (h d) -> p k h d", d=HD))
                        return f

                    units.append(w_load("k", wk_d))
                    units.append(w_load("q", wq_d))
                    units.append(w_load("v", wv_d))
                    for s5 in range(4):
                        units.append(k_chain(s5))
                    for s5 in range(2):
                        units.append(q_chain(s5))
                    for g in range(4):
                        units.append(v_group(g))
                    return units

                def make_attn_units(p, state):
                    """Attention stream for pair p: per head 16 x (S, exp,
                    PV one kc behind) + normalize, then attnT transposes for
                    this pair's din columns."""
                    kTh, qTh, vONp = state[p]
                    units = []

                    def pv_step(kc, hh, pv_tiles, es_tiles):
                        es = es_tiles.pop(kc)
                        for qc in range(8):
                            # one accumulation buffer per psum bank: only
                            # the bank's first chain may zero it (start);
                            # siblings accumulate onto the zeroed buffer.
                            nc.tensor.matmul(
                                pv_tiles[qc // 4][:, qc % 4, :],
                                lhsT=es[:, qc * P:(qc + 1) * P],
                                rhs=vONp[:, kc, hh, :],
                                start=(kc == 0 and qc % 4 == 0),
                                stop=(kc == NKC - 1),
                                skip_group_check=True)

                    for hh in range(2):
                        kTp, qTp = kTh[hh], qTh[hh]
                        pv_tiles = []
                        es_tiles = {}

                        def head_setup(pv_tiles=pv_tiles):
                            for _ in range(2):
                                pv_tiles.append(pvps.tile(
                                    [P, 4, HD + 1], F32, name="pv", tag="pv"))

                        def kc_unit(kc, hh=hh, kTp=kTp, qTp=qTp,
                                    pv_tiles=pv_tiles, es_tiles=es_tiles):
                            def f():
                                ps = scoreps.tile([P, NQ], F32, name="sps",
                                                  tag="sc")
                                for qc2 in range(2):
                                    nc.tensor.matmul(
                                        ps[:, qc2 * 512:(qc2 + 1) * 512],
                                        lhsT=kTp[:, kc * P:(kc + 1) * P],
                                        rhs=qTp[:,
                                                qc2 * 512:(qc2 + 1) * 512])
                                es = esp.tile([P, NQ], BF16, name="es",
                                              tag="es")
                                nc.scalar.activation(es[:], ps[:], EXP,
                                                     scale=SCALE)
                                es_tiles[kc] = es
                                if kc > 0:
                                    # PV runs one kc behind its exp so the
                                    # PE never waits on ACT just-in-time
                                    pv_step(kc - 1, hh, pv_tiles, es_tiles)
                            return f

                        def pv_tail(hh=hh, pv_tiles=pv_tiles,
                                    es_tiles=es_tiles):
                            def f():
                                pv_step(NKC - 1, hh, pv_tiles, es_tiles)
                            return f

                        def norm_unit(half, p=p, hh=hh, pv_tiles=pv_tiles):
                            def f():
                                pv = pv_tiles[half]
                                rec = recp.tile([P, 4, 1], F32, name="rec",
                                                tag="rec")
                                nc.vector.reciprocal(
                                    rec[:], pv[:, :, HD:HD + 1])
                                for qc4 in range(4):
                                    qc = half * 4 + qc4
                                    nc.vector.tensor_scalar(
                                        attnN[:, qc,
                                              (2 * p + hh) * HD:
                                              (2 * p + hh + 1) * HD],
                                        pv[:, qc4, 0:HD],
                                        rec[:, qc4, :], None, MULT)
                            return f

                        units.append(head_setup)
                        for kc in range(NKC):
                            units.append(kc_unit(kc))
                        units.append(pv_tail())
                        units.append(norm_unit(0))
                        units.append(norm_unit(1))

                    def at_unit(qg, p=p):
                        def f():
                            tp = projps.tile([P, 4, P], BF16, name="tp",
                                             tag="pp")
                            for q4 in range(4):
                                qc = qg * 4 + q4
                                nc.tensor.transpose(
                                    tp[:, q4, :],
                                    attnN[:, qc, p * P:(p + 1) * P],
                                    ident16[:])
                            nc.vector.tensor_copy(
                                attnT[:, p, qg * 512:(qg + 1) * 512]
                                .rearrange("p (q c) -> p q c", c=P),
                                tp[:])
                        return f

                    units.append(at_unit(0))
                    units.append(at_unit(1))
                    return units

                # ---------- pipelined emission ----------
                state = {}
                zt_units = [zt_unit(ch) for ch in range(16)]
                for p in range(NP + 1):
                    proj_units = make_proj_units(p, state) if p < NP else []
                    attn_units = make_attn_units(p - 1, state) if p > 0 else []
                    if not attn_units:
                        # pair 0 fill: z chunks interleaved with pair-0
                        # chains in dependency order (chain s5 needs z
                        # chunks 4*s5..4*s5+3)
                        wl, ch_u = proj_units[0:3], proj_units[3:]
                        K, Q, V = ch_u[0:4], ch_u[4:6], ch_u[6:10]
                        order = ([wl[0]] + zt_units[0:4] + [K[0], wl[1]] +
                                 zt_units[4:8] + [Q[0], Q[1], wl[2]] +
                                 zt_units[8:12] + [K[1], V[0]] +
                                 zt_units[12:16] +
                                 [K[2], V[1], K[3], V[2], V[3]])
                        for u in order:
                            u()
                        continue
                    # interleave: sprinkle proj units evenly through the
                    # (longer) attention stream
                    na, npj = len(attn_units), len(proj_units)
                    pi = 0
                    for i, u in enumerate(attn_units):
                        u()
                        want = ((i + 1) * npj) // na
                        while pi < want:
                            proj_units[pi]()
                            pi += 1
                    while pi < npj:
                        proj_units[pi]()
                        pi += 1
                    if p == 5:
                        # preload + cast w_o while attention still runs
                        for half in range(2):
                            stg = wostgp.tile([P, DC // 2, D], F32,
                                              name="wostg", tag="wostg")
                            nc.sync.dma_start(
                                stg[:],
                                wo_d[half * 512:(half + 1) * 512, :]
                                .rearrange("(c p) o -> p c o", p=P))
                            nc.vector.tensor_copy(
                                wo16[:, half * 4:(half + 1) * 4, :], stg[:])

            # ---------------- tail: final projection ----------
            with tc.tile_pool(name="ot", bufs=4) as outp, \
                 tc.tile_pool(name="psf", bufs=2, space="PSUM") as fpp:
                for qc in range(NQ // P):
                    for oc2 in range(2):
                        po = fpp.tile([P, 512], F32, name="po", tag="po")
                        for dc in range(DC):
                            nc.tensor.matmul(
                                po[:],
                                lhsT=attnT[:, dc, qc * P:(qc + 1) * P],
                                rhs=wo16[:, dc, oc2 * 512:(oc2 + 1) * 512],
                                start=(dc == 0), stop=(dc == DC - 1))
                        ot = outp.tile([P, 512], F32, name="ot", tag="ot")
                        nc.vector.tensor_tensor(
                            ot[:], po[:], bias_bc[:, oc2 * 512:(oc2 + 1) * 512],
                            ADD)
                        nc.sync.dma_start(
                            out_d[qc * P:(qc + 1) * P,
                                  oc2 * 512:(oc2 + 1) * 512], ot[:])

    nc.compile()
    return nc


_NC_CACHE = None


def _get_nc():
    global _NC_CACHE
    if _NC_CACHE is None:
        _NC_CACHE = _build()
    return _NC_CACHE


def _run(z, w_q, w_k, w_v, w_o, b_o, **spmd_kwargs):
    z = np.ascontiguousarray(np.asarray(z, dtype=np.float32))
    w_q = np.ascontiguousarray(np.asarray(w_q, dtype=np.float32))
    w_k = np.ascontiguousarray(np.asarray(w_k, dtype=np.float32))
    w_v = np.ascontiguousarray(np.asarray(w_v, dtype=np.float32))
    w_o = np.ascontiguousarray(np.asarray(w_o, dtype=np.float32))
    b_o = np.ascontiguousarray(np.asarray(b_o, dtype=np.float32))
    assert z.shape == (B, N, D)

    if not spmd_kwargs.get("trace"):
        # A stray BASS_TRACE in the environment would route through the NTFF
        # hook (absent in this image) and crash; force the no-trace path.
        os.environ["BASS_NEVER_TRACE"] = "1"

    nc = _get_nc()
    in_maps = []
    for c in range(N_CORES):
        b = c // 2
        off = (c % 2) * NQ
        zc = np.ascontiguousarray(np.concatenate([z[b, off:], z[b, :off]], axis=0))
        in_maps.append({"z": zc, "w_q": w_q, "w_k": w_k, "w_v": w_v,
                        "w_o": w_o, "b_o": b_o})

    res = run_bass_kernel_spmd(nc, in_maps, core_ids=list(range(N_CORES)),
                               **spmd_kwargs)
    out = np.empty((B, N, D), dtype=np.float32)
    for c in range(N_CORES):
        b = c // 2
        off = (c % 2) * NQ
        out[b, off:off + NQ, :] = res.results[c]["out"]
    return out, res


def kernel(z, w_q, w_k, w_v, w_o, b_o):
    out, _ = _run(z, w_q, w_k, w_v, w_o, b_o)
    return out
